# revision 28
# baseline (speedup 1.0000x reference)
"""Trainium2 Bass kernel for masked BasicBlock (grouped conv3x3 -> BN -> ReLU
-> masked grouped conv3x3 -> BN -> +residual -> ReLU).

Strategy: data-parallel over batch across 8 NeuronCores (2 images/core);
grouped conv mapped to accumulating matmuls over a zero-padded SBUF image
layout; global training-mode BN stats via per-pair 1KB AllReduces pipelined
against the other channel-pair's conv so their latency is hidden.

v2 changes vs the original baseline:
  - Input DMA batching: one big const tensor (weights+masks), per-C-tile
    image loads, and the row-shifted duplicate built on-device via
    SBUF->SBUF DMA instead of a host-doubled upload (dma_start issue costs
    ~600ns each on the SP queue regardless of size; fewer+bigger wins).
  - Pair-major loop order with 4 split AllReduces (one per conv x channel
    pair) so each collective overlaps the other pair's compute.
  - Weight-grouped matmul order: all matmuls K=128/M=64 col-tile pairs;
    same weights reused across the 7 row-tiles so LDWEIGHTS leaves the
    inner loop; per-col-half start=True opens each PSUM bank half.
  - bf16 output DMA (host upcasts), fuse arithmetic in bf16.

Conv mapping per 128-channel group-pair, per 8-row output tile (N=448):
  - input tiles C_g = [ci(64) ; ci(64) shifted +1 row] so one K=128 matmul
    covers two dy taps at once; the third dy tap is a K=128 matmul with
    zero weights in rows 64:128 (avoids a PE tiling-mode switch).
  - the two groups' M=64 matmuls are issued at col positions 0/64 so they
    execute concurrently on disjoint PE array columns.

Self-contained: hardcodes shapes from the problem spec.
"""
from contextlib import ExitStack

import numpy as np
import ml_dtypes

import concourse.bacc as bacc
import concourse.bass as bass
import concourse.mybir as mybir
from concourse.tile import TileContext
from concourse.bass_utils import run_bass_kernel_spmd

F32 = mybir.dt.float32
BF16 = mybir.dt.bfloat16
AF = mybir.ActivationFunctionType
ALU = mybir.AluOpType

N_CORES = 8
IMG = 2              # images per core
CIN = 256
G = 4
PAIRS = 2            # pairs of channel groups (128 ch each)
H = W = 56
PH, PW = 59, 58      # padded rows / cols (rows 0,57,58 and cols 0,57 zero)
PADN = PH * PW       # 3422
INT0 = PW            # flat offset of padded row 1
INTN = 56 * PW       # 3248: rows 1..56, all 58 cols
ROWT = 7             # 8-row output tiles per image
TN = 8 * W           # 448 pixels per psum tile
HW = H * W           # 3136
EPS = 1e-5
N_CORE_CNT = IMG * H * W
N_TOT = 16 * H * W

WCOLS = 2 * PAIRS * 6 * 2 * 64     # 3072
MRCOLS = IMG * PAIRS * 7 * PW      # 1624
CBCOLS = WCOLS + MRCOLS

_prog_cache = {}


def _sub_ap(base, off, dims):
    """Custom free-dim access pattern on an existing AP (keeps partition dim)."""
    return bass.AP(
        tensor=base.tensor,
        offset=base.offset + off,
        ap=[list(base.ap[0])] + [list(d) for d in dims],
    )


def _build_program():
    nc = bacc.Bacc(num_devices=N_CORES)

    # base (masked, padded) conv1 input per (pair, img, g2): [64, PADN]
    xm_d = nc.dram_tensor("xm", [PAIRS, IMG, 2, 64, PADN], BF16,
                          kind="ExternalInput")
    # all conv weights (lhsT layout) + expanded mask rows, partition-major
    cb_d = nc.dram_tensor("cbig", [128, CBCOLS], BF16, kind="ExternalInput")
    gb_d = nc.dram_tensor("gb", [128, 8], F32, kind="ExternalInput")
    xr_d = nc.dram_tensor("xres", [IMG, CIN, HW], BF16, kind="ExternalInput")
    y_d = nc.dram_tensor("y", [IMG, CIN, HW], BF16, kind="ExternalOutput")

    with TileContext(nc) as tc, ExitStack() as es:
        consts = es.enter_context(tc.tile_pool(name="consts", bufs=1))
        small = es.enter_context(tc.tile_pool(name="small", bufs=24))
        cp = es.enter_context(tc.tile_pool(name="cp", bufs=11))
        crp = es.enter_context(tc.tile_pool(name="crp", bufs=8))
        xrp = es.enter_context(tc.tile_pool(name="xrp", bufs=4))
        obp = es.enter_context(tc.tile_pool(name="obp", bufs=4))
        psp = es.enter_context(tc.tile_pool(name="psp", bufs=8, space="PSUM"))
        drp = es.enter_context(tc.tile_pool(name="drp", bufs=1, space="DRAM"))

        # ---- consts to SBUF (2 DMAs) ----
        cb_sb = consts.tile([128, CBCOLS], BF16, tag="cb", name="cb")
        nc.sync.dma_start(out=cb_sb[:], in_=cb_d[:])
        gb_sb = consts.tile([128, 8], F32, tag="gbs", name="gbs")
        nc.sync.dma_start(out=gb_sb[:], in_=gb_d[:])

        def wsl(conv, pair, grp, g2):
            off = (((conv * 2 + pair) * 6 + grp) * 2 + g2) * 64
            return cb_sb[:, off:off + 64]

        def msl(img, pair):
            off = WCOLS + (img * 2 + pair) * 7 * PW
            return cb_sb[:, off:off + 7 * PW]

        def gamall(conv):
            return gb_sb[:, conv * 4:conv * 4 + 2]

        def betall(conv):
            return gb_sb[:, conv * 4 + 2:conv * 4 + 4]

        eps_sb = consts.tile([128, 1], F32, tag="eps", name="eps")
        nc.vector.memset(eps_sb[:], EPS)

        # persistent relu-scratch tiles; boundary cols zeroed once at start
        # (as pool tiles their per-use memsets landed late in the DVE FIFO
        # and stalled the conv2 pair transition by ~5us)
        yts = [consts.tile([128, PADN], BF16, tag=f"yt{i}", name=f"yt{i}")
               for i in range(3)]
        for t in yts:
            nc.vector.memset(_sub_ap(t[:], 0, [[PW, PH]]), 0)
            nc.vector.memset(_sub_ap(t[:], PW - 1, [[PW, PH]]), 0)
        yt_ctr = [0]

        stats_sb = {
            (c, p): consts.tile([128, IMG * ROWT * 6], F32, tag=f"st{c}{p}",
                                name=f"st{c}{p}")
            for c in range(2) for p in range(PAIRS)
        }
        # a,b per conv: [128, 2] (col = pair)
        ab_sb = {}
        for conv in range(2):
            ab_sb[conv] = (consts.tile([128, 2], F32, tag=f"a{conv}",
                                       name=f"a{conv}"),
                           consts.tile([128, 2], F32, tag=f"b{conv}",
                                       name=f"b{conv}"))

        def asl(conv, pair):
            return ab_sb[conv][0][:, pair:pair + 1]

        def bsl(conv, pair):
            return ab_sb[conv][1][:, pair:pair + 1]

        cc_in = {c: drp.tile([128, 4], F32, tag=f"ccin{c}", name=f"ccin{c}")
                 for c in range(4)}
        cc_out = {c: drp.tile([128, 4], F32, addr_space="Shared",
                              tag=f"ccout{c}", name=f"ccout{c}") for c in range(4)}

        craw = {}      # (conv, pair, img) -> [128, HW] bf16 conv outputs

        # ---------------- helpers ----------------
        def load_conv1_pair(pair):
            """DMA base halves for both imgs (split across the two hwdge
            queues), then dup-shift on DVE (a DVE copy is ~4x faster than an
            SBUF->SBUF DMA for these 420KB tiles and keeps the queues free)."""
            Cs = {}
            for img in range(IMG):
                for g2 in range(2):
                    eng = nc.sync if g2 == 0 else nc.scalar
                    C = cp.tile([128, PADN], BF16, tag="C", name="C")
                    eng.dma_start(out=C[0:64, :], in_=xm_d[pair, img, g2])
                    Cs[(img, g2)] = C
            for img in range(IMG):
                for g2 in range(2):
                    C = Cs[(img, g2)]
                    # rows 0..57 of shifted half <- base rows 1..58
                    nc.vector.tensor_scalar_mul(
                        C[64:128, 0:58 * PW], C[0:64, PW:PADN], 1.0)
                    nc.vector.memset(C[64:128, 58 * PW:PADN], 0)
            return Cs

        def conv_block(conv, pair, img, Cs, craw_t, st):
            psums = [psp.tile([128, TN], F32, tag="ps", name="ps")
                     for _ in range(ROWT)]
            for grp in range(6):
                dx = grp % 3
                roff = 2 * PW if grp >= 3 else 0
                for t in range(ROWT):
                    off = (8 * t) * PW + roff + dx
                    for g2 in range(2):
                        rhs = _sub_ap(Cs[g2][:], off, [[PW, 8], [1, W]])
                        nc.tensor.matmul(
                            psums[t][64 * g2:64 * (g2 + 1), :],
                            wsl(conv, pair, grp, g2), rhs,
                            start=(grp == 0), stop=(grp == 5),
                            tile_position=(0, 64 * g2))
            for t in range(ROWT):
                # stats straight from PSUM so the AllReduce trigger never
                # waits on the ACT evacuation chain
                nc.vector.bn_stats(
                    out=st[:, (img * ROWT + t) * 6:(img * ROWT + t + 1) * 6],
                    in_=psums[t][:])
                seg = craw_t[:, TN * t:TN * (t + 1)]
                nc.scalar.activation(out=seg, in_=psums[t][:], func=AF.Copy)

        def _sq_cols(sq, c0, st):
            """sum and sumsq of one stats tile into sq cols c0 / c0+2."""
            mv = small.tile([128, 2], F32, tag="mv", name="mv")
            nc.vector.bn_aggr(out=mv[:],
                              in_=st[:].rearrange("p (n s) -> p n s", s=6))
            nc.vector.tensor_scalar_mul(sq[:, c0:c0 + 1], mv[:, 0:1],
                                        float(N_CORE_CNT))
            msq = small.tile([128, 1], F32, tag="msq", name="msq")
            nc.vector.tensor_mul(msq[:], mv[:, 0:1], mv[:, 0:1])
            nc.vector.tensor_add(msq[:], msq[:], mv[:, 1:2])
            nc.vector.tensor_scalar_mul(sq[:, c0 + 2:c0 + 3], msq[:],
                                        float(N_CORE_CNT))

        def _bnmath(sq2, cols, a_ap, b_ap, gam_ap, bet_ap):
            """sq2 [sums | sumsqs] (width cols) -> a,b [128, cols]."""
            mu = small.tile([128, 2], F32, tag="mu", name="mu")[:, 0:cols]
            nc.vector.tensor_scalar_mul(mu, sq2[:, 0:cols], 1.0 / N_TOT)
            ex2 = small.tile([128, 2], F32, tag="ex2", name="ex2")[:, 0:cols]
            nc.vector.tensor_scalar_mul(ex2, sq2[:, 2:2 + cols], 1.0 / N_TOT)
            msq2 = small.tile([128, 2], F32, tag="msq2", name="msq2")[:, 0:cols]
            nc.vector.tensor_mul(msq2, mu, mu)
            nc.vector.tensor_sub(ex2, ex2, msq2)               # biased var
            sd = small.tile([128, 2], F32, tag="sd", name="sd")[:, 0:cols]
            nc.scalar.activation(out=sd, in_=ex2, func=AF.Sqrt,
                                 bias=eps_sb[:])
            rstd = small.tile([128, 2], F32, tag="rstd", name="rstd")[:, 0:cols]
            nc.vector.reciprocal(out=rstd, in_=sd)
            nc.vector.tensor_mul(a_ap, gam_ap, rstd)
            t3 = small.tile([128, 2], F32, tag="t3", name="t3")[:, 0:cols]
            nc.vector.tensor_mul(t3, a_ap, mu)
            nc.vector.tensor_sub(b_ap, bet_ap, t3)

        sq1c = consts.tile([128, 4], F32, tag="sq1c", name="sq1c")

        def sr1_trigger():
            """conv1: one merged AllReduce for both pairs (the CC stream is
            boot-gated until ~75us anyway, so splitting buys nothing and a
            second op costs ~15us of serial stream time). Pair 0's columns
            were aggregated right after its blocks."""
            sq = sq1c
            _sq_cols(sq, 1, stats_sb[(0, 1)])
            nc.sync.dma_start(out=cc_in[0][:], in_=sq[:])
            nc.gpsimd.collective_compute(
                "AllReduce", ALU.add,
                replica_groups=[list(range(N_CORES))],
                ins=[cc_in[0][:]], outs=[cc_out[0][:]],
            )

        def sr1_finish():
            sq2 = small.tile([128, 4], F32, tag="sq24", name="sq24")
            nc.sync.dma_start(out=sq2[:], in_=cc_out[0][:])
            a_t, b_t = ab_sb[0]
            _bnmath(sq2, 2, a_t[:], b_t[:], gamall(0), betall(0))

        def sr_trigger(conv, pair):
            """conv2: per-pair AllReduce so pair 0's overlaps pair 1's conv."""
            k = 2 + pair
            sq = small.tile([128, 4], F32, tag="sq", name="sq")
            _sq_cols(sq, 0, stats_sb[(conv, pair)])
            nc.sync.dma_start(out=cc_in[k][:], in_=sq[:])
            nc.gpsimd.collective_compute(
                "AllReduce", ALU.add,
                replica_groups=[list(range(N_CORES))],
                ins=[cc_in[k][:]], outs=[cc_out[k][:]],
            )

        def sr_finish(conv, pair):
            k = 2 + pair
            sq2 = small.tile([128, 4], F32, tag="sq2", name="sq2")
            nc.sync.dma_start(out=sq2[:], in_=cc_out[k][:])
            _bnmath(sq2, 1, asl(conv, pair), bsl(conv, pair),
                    gamall(conv)[:, pair:pair + 1],
                    betall(conv)[:, pair:pair + 1])

        def prep_img(pair, img, dve_shift):
            """Build one image's conv2 input tiles: relu(a1*c1+b1)*mask in
            dup-shifted padded layout. dve_shift picks DVE muls (low
            latency) vs a sync-queue DMA (keeps DVE free) for the
            row-shifted duplicate half."""
            out = {}
            yt = yts[yt_ctr[0] % 3]
            yt_ctr[0] += 1
            nc.scalar.activation(
                out=_sub_ap(yt[:], PW + 1, [[PW, H], [1, W]]),
                in_=craw[(0, pair, img)][:],
                func=AF.Relu,
                bias=bsl(0, pair),
                scale=asl(0, pair),
            )
            for g2 in range(2):
                C = cp.tile([128, PADN], BF16, tag="C", name="C")
                mask_ap = _sub_ap(msl(img, pair)[64 * g2:64 * (g2 + 1)], 0,
                                  [[PW, 7], [0, 8], [1, PW]])
                ysrc = yt[64 * g2:64 * (g2 + 1), :]
                nc.vector.memset(C[0:64, 0:PW], 0)
                nc.vector.memset(C[0:64, 57 * PW:PADN], 0)
                nc.vector.tensor_mul(
                    C[0:64, INT0:INT0 + INTN],
                    ysrc[:, INT0:INT0 + INTN], mask_ap)
                if dve_shift:
                    nc.vector.memset(C[64:128, 56 * PW:PADN], 0)
                    nc.vector.tensor_mul(
                        C[64:128, 0:INTN],
                        ysrc[:, INT0:INT0 + INTN], mask_ap)
                else:
                    nc.sync.dma_start(out=C[64:128, 0:58 * PW],
                                      in_=C[0:64, PW:PADN])
                    nc.vector.memset(C[64:128, 58 * PW:PADN], 0)
                out[(img, g2)] = C
            return out

        def fuse_pair(pair, xr_tiles):
            """relu(a2*c2 + x + b2) -> y (bf16)."""
            HNW = HW // 2
            for img in range(IMG):
                for half in range(2):
                    seg = slice(HNW * half, HNW * (half + 1))
                    ob = obp.tile([128, HNW], BF16, tag="ob", name="ob")
                    nc.vector.scalar_tensor_tensor(
                        out=ob[:],
                        in0=craw[(1, pair, img)][:, seg],
                        scalar=asl(1, pair),
                        in1=xr_tiles[(img, pair)][:, seg],
                        op0=ALU.mult, op1=ALU.add)
                    nc.scalar.activation(out=ob[:], in_=ob[:], func=AF.Relu,
                                         bias=bsl(1, pair))
                    nc.scalar.dma_start(
                        out=y_d[img, 128 * pair:128 * (pair + 1), seg],
                        in_=ob[:])

        # ---------------- program ----------------
        Cs1 = {0: load_conv1_pair(0)}
        for img in range(IMG):
            craw[(0, 0, img)] = crp.tile([128, HW], BF16, tag="cr", name="cr")
            conv_block(0, 0, img, [Cs1[0][(img, 0)], Cs1[0][(img, 1)]],
                       craw[(0, 0, img)], stats_sb[(0, 0)])
        _sq_cols(sq1c, 0, stats_sb[(0, 0)])
        Cs1[1] = load_conv1_pair(1)
        for img in range(IMG):
            craw[(0, 1, img)] = crp.tile([128, HW], BF16, tag="cr", name="cr")
            conv_block(0, 1, img, [Cs1[1][(img, 0)], Cs1[1][(img, 1)]],
                       craw[(0, 1, img)], stats_sb[(0, 1)])
        # residual prefetch on the scalar queue
        xr_tiles = {}
        for img in range(IMG):
            for pair in range(PAIRS):
                xr = xrp.tile([128, HW], BF16, tag="xr", name="xr")
                nc.scalar.dma_start(
                    out=xr[:],
                    in_=xr_d[img, 128 * pair:128 * (pair + 1)])
                xr_tiles[(img, pair)] = xr
        sr1_trigger()
        sr1_finish()
        C2 = {}
        C2.update(prep_img(0, 0, dve_shift=True))
        C2.update(prep_img(0, 1, dve_shift=False))
        C2p1 = {}
        # pair-1 prep interleaves with the pair-0 conv2 blocks so the DVE
        # order is [p1i0 muls, p0i0 stats, p1i1 muls, p0i1 stats, aggr]
        C2p1.update(prep_img(1, 0, dve_shift=True))
        craw[(1, 0, 0)] = crp.tile([128, HW], BF16, tag="cr", name="cr")
        conv_block(1, 0, 0, [C2[(0, 0)], C2[(0, 1)]],
                   craw[(1, 0, 0)], stats_sb[(1, 0)])
        C2p1.update(prep_img(1, 1, dve_shift=False))
        craw[(1, 0, 1)] = crp.tile([128, HW], BF16, tag="cr", name="cr")
        conv_block(1, 0, 1, [C2[(1, 0)], C2[(1, 1)]],
                   craw[(1, 0, 1)], stats_sb[(1, 0)])
        sr_trigger(1, 0)
        for img in range(IMG):
            craw[(1, 1, img)] = crp.tile([128, HW], BF16, tag="cr", name="cr")
            conv_block(1, 1, img, [C2p1[(img, 0)], C2p1[(img, 1)]],
                       craw[(1, 1, img)], stats_sb[(1, 1)])
        sr_finish(1, 0)
        fuse_pair(0, xr_tiles)
        sr_trigger(1, 1)
        sr_finish(1, 1)
        fuse_pair(1, xr_tiles)

    nc.compile()
    return nc


def _pack_wall(w1, w2):
    """w [256,64,3,3] f32 x2 -> [128, WCOLS] bf16 lhsT layout."""
    wall = np.zeros([128, WCOLS], np.float32)
    for conv, w in enumerate((w1, w2)):
        w = np.asarray(w, np.float32)
        for pair in range(PAIRS):
            for grp in range(6):
                dx = grp % 3
                for g2 in range(2):
                    g = 2 * pair + g2
                    blk = w[64 * g:64 * (g + 1)]        # [64co, 64ci, 3, 3]
                    off = (((conv * 2 + pair) * 6 + grp) * 2 + g2) * 64
                    if grp < 3:
                        wall[0:64, off:off + 64] = blk[:, :, 0, dx].T
                        wall[64:128, off:off + 64] = blk[:, :, 1, dx].T
                    else:
                        wall[0:64, off:off + 64] = blk[:, :, 2, dx].T
    return wall


def _pack_mrows(mask_core):
    """mask [IMG,4,7,7] -> [128, MRCOLS] bf16 (cols 0,57 of each row zero)."""
    mexp = np.repeat(mask_core, 8, axis=-1)         # [IMG,4,7,56]
    mr = np.zeros([128, IMG, PAIRS, 7, PW], np.float32)
    for img in range(IMG):
        for pair in range(PAIRS):
            for g2 in range(2):
                g = 2 * pair + g2
                mr[64 * g2:64 * (g2 + 1), img, pair, :, 1:57] = \
                    mexp[img, g][None, :, :]
    return mr.reshape(128, MRCOLS)


def _expand_mask_full(mask):
    """mask [N,4,7,7] -> [N,256,56,56] nearest-upsampled, channel-repeated."""
    m = np.repeat(np.repeat(mask, 8, axis=2), 8, axis=3)
    return np.repeat(m, CIN // G, axis=1)


def make_in_maps(x, mask, w1, gamma1, beta1, w2, gamma2, beta2):
    x = np.asarray(x, np.float32)
    mask = np.asarray(mask, np.float32)
    bf = ml_dtypes.bfloat16
    xm_full = x * _expand_mask_full(mask)
    wall = _pack_wall(w1, w2)

    # cols: conv*4 + [gamma_p0, gamma_p1, beta_p0, beta_p1]
    gb = np.zeros([128, 8], np.float32)
    for conv, (gmm, bt) in enumerate(((gamma1, beta1), (gamma2, beta2))):
        gmm = np.asarray(gmm, np.float32)
        bt = np.asarray(bt, np.float32)
        for pair in range(PAIRS):
            sl = slice(128 * pair, 128 * (pair + 1))
            gb[:, conv * 4 + pair] = gmm[sl]
            gb[:, conv * 4 + 2 + pair] = bt[sl]

    in_maps = []
    for core in range(N_CORES):
        sl = slice(IMG * core, IMG * (core + 1))
        xmc = xm_full[sl]                            # [IMG,256,56,56]
        xm = np.zeros([PAIRS, IMG, 2, 64, PH, PW], np.float32)
        for pair in range(PAIRS):
            for g2 in range(2):
                g = 2 * pair + g2
                xm[pair, :, g2, :, 1:57, 1:57] = xmc[:, 64 * g:64 * (g + 1)]
        cbig = np.concatenate([wall, _pack_mrows(mask[sl])], axis=1)
        in_maps.append({
            "xm": xm.reshape(PAIRS, IMG, 2, 64, PADN).astype(bf),
            "cbig": cbig.astype(bf),
            "gb": gb,
            "xres": np.ascontiguousarray(
                x[sl].reshape(IMG, CIN, HW).astype(bf)),
        })
    return in_maps


def kernel(**inputs):
    if "nc" not in _prog_cache:
        _prog_cache["nc"] = _build_program()
    nc = _prog_cache["nc"]
    in_maps = make_in_maps(**inputs)
    res = run_bass_kernel_spmd(nc, in_maps, list(range(N_CORES)))
    y = np.concatenate(
        [np.asarray(res.results[i]["y"], np.float32).reshape(IMG, CIN, H, W)
         for i in range(N_CORES)], axis=0)
    return y
